# revision 10
# baseline (speedup 1.0000x reference)
"""Trainium2 Bass kernel for nn_DDI_3367254360364.

Data-parallel over batch B=128 across 8 cores (16 each). Per-step BatchNorm
statistics are exchanged with 49KB fp32 AllReduces (small collectives
pipeline for ~free on this fabric; large fused ones run at ~38MB/s, so the
1.3MB outer-BN stats exchange is split into 28 per-window collectives).
Layout per core: 4 f-chunks of [128f, 16b, 336t] resident in SBUF; the
window recurrence writes outputs in place.

agg einsum ('bfp,qp->bfq') runs on the PE via transpose -> block-diagonal
(I_8 (x) agg_w) matmul with 12 extra bias rows carrying the BN1 additive
term -> transpose back. fc1/fc2 are bf16 matmuls with pre-transposed weights.

The compiled NEFF + jitted PJRT executable + device-resident weights are
cached at module level, so repeat kernel() calls ship only x in / y out.

_build(reps=R, timing=True) emits a timing replica: the full body executed
R times back-to-back against DRAM scratch (same DMA patterns), tiny I/O;
test.py uses (wall(R) - wall(1)) / (R - 1) as an axon-noise-immune estimate
of one execution's device time.
"""

import sys

sys.path.insert(0, "/opt/trn_rl_repo")

from contextlib import ExitStack
import time as _time

import numpy as np
import ml_dtypes

import jax
from jax.sharding import Mesh, PartitionSpec, NamedSharding
from jax.experimental.shard_map import shard_map

from concourse import bass, bacc, mybir, tile
from concourse.bass2jax import (
    _bass_exec_p,
    install_neuronx_cc_hook,
    partition_id_tensor,
)

F32 = mybir.dt.float32
BF16 = mybir.dt.bfloat16

B, F, T = 128, 512, 336
PATCH = 12
NCORES = 8
BL = B // NCORES          # 16 local batch
NW = T // PATCH           # 28 windows
EPS = 1e-5
ALPHA = 0.5
NC_F = F // 128           # 4 f-chunks
BT = BL * T               # 5376 free elems per x chunk tile
LAST_RUN_WALL = None      # wall seconds of the last run (for test.py timing)


def _build(nc: bass.Bass, nwin: int, reps: int = 1, timing: bool = False):
    """Emit the kernel IR for `nwin` windows (nwin=NW for the real thing)."""
    if not timing:
        x_d = nc.declare_dram_parameter("x", [BL, F, T], F32, isOutput=False)
    w1t_d = nc.declare_dram_parameter("w1t", [F, F], BF16, isOutput=False)
    w2t_d = nc.declare_dram_parameter("w2t", [F, F], BF16, isOutput=False)
    kron_d = nc.declare_dram_parameter("kron", [108, 96], BF16, isOutput=False)
    awt_d = nc.declare_dram_parameter("awt", [PATCH, PATCH], BF16, isOutput=False)
    idf_d = nc.declare_dram_parameter("idf", [128, 128], F32, isOutput=False)
    idb_d = nc.declare_dram_parameter("idb", [128, 128], BF16, isOutput=False)
    if timing:
        y_d = nc.declare_dram_parameter("y", [128, 16], F32, isOutput=True)
    else:
        y_d = nc.declare_dram_parameter("y", [BL, F, T], F32, isOutput=True)

    with tile.TileContext(nc) as tc, ExitStack() as ctx:
        main = ctx.enter_context(tc.tile_pool(name="main", bufs=1))
        wk = ctx.enter_context(tc.tile_pool(name="wk", bufs=2))
        sqp = ctx.enter_context(tc.tile_pool(name="sqp", bufs=1))
        stp = ctx.enter_context(tc.tile_pool(name="stp", bufs=1))
        dram = ctx.enter_context(tc.tile_pool(name="dram", bufs=2, space="DRAM"))
        pSm = ctx.enter_context(tc.tile_pool(name="pSm", bufs=1, space="PSUM"))
        pT = ctx.enter_context(tc.tile_pool(name="pT", bufs=3, space="PSUM"))
        pBig = ctx.enter_context(tc.tile_pool(name="pBig", bufs=4, space="PSUM"))

        # ---- constants / weights into SBUF ----
        w1t = [main.tile([128, F], BF16, tag=f"w1t{k}", name=f"w1t{k}") for k in range(NC_F)]
        w2t = [main.tile([128, F], BF16, tag=f"w2t{k}", name=f"w2t{k}") for k in range(NC_F)]
        for k in range(NC_F):
            nc.sync.dma_start(w1t[k][:], w1t_d[k * 128:(k + 1) * 128, :])
            nc.sync.dma_start(w2t[k][:], w2t_d[k * 128:(k + 1) * 128, :])
        kron = main.tile([108, 96], BF16, tag="kron", name="kron")
        nc.sync.dma_start(kron[:], kron_d[:])
        awt = main.tile([PATCH, PATCH], BF16, tag="awt", name="awt")
        nc.sync.dma_start(awt[:], awt_d[:])
        idf = main.tile([128, 128], F32, tag="idf", name="idf")
        nc.sync.dma_start(idf[:], idf_d[:])
        idb = main.tile([128, 128], BF16, tag="idb", name="idb")
        nc.sync.dma_start(idb[:], idb_d[:])
        epsb = main.tile([128, 1], F32, tag="epsb", name="epsb")
        nc.vector.memset(epsb[:], EPS)

        xb = [main.tile([128, BL, T], F32, tag=f"xb{c}", name=f"xb{c}") for c in range(NC_F)]

        if timing:
            # DRAM scratch stands in for the x/y parameters so each rep still
            # pays the real load/store DMA patterns.
            x_src = dram.tile([BL, F, T], F32, tag="xscr", name="xscr", bufs=1)
            y_dst = dram.tile([BL, F, T], F32, tag="yscr", name="yscr", bufs=1)
            for c in range(NC_F):
                nc.vector.memset(xb[c][:], 0.25 + 0.01 * c)
                nc.sync.dma_start(
                    x_src[:, c * 128:(c + 1) * 128, :].rearrange("b f t -> f b t"),
                    xb[c][:])
        else:
            x_src, y_dst = x_d, y_d

        def scales(S, n, tagp):
            """S: [128, NC_F, 2, G] sums/sumsqs -> (s, t) each [128, NC_F, G]."""
            G = S.shape[3]
            nm = stp.tile([128, NC_F, G], F32, tag=tagp + "nm")
            q = stp.tile([128, NC_F, G], F32, tag=tagp + "q")
            v = stp.tile([128, NC_F, G], F32, tag=tagp + "v")
            nc.vector.tensor_scalar_mul(nm[:], S[:, :, 0, :], -1.0 / n)
            nc.vector.tensor_scalar_mul(q[:], S[:, :, 1, :], 1.0 / n)
            nc.vector.tensor_tensor(v[:], nm[:], nm[:], op=mybir.AluOpType.mult)
            nc.vector.tensor_tensor(v[:], q[:], v[:], op=mybir.AluOpType.subtract)
            nc.scalar.activation(v[:], v[:], mybir.ActivationFunctionType.Sqrt,
                                 bias=epsb[:])
            nc.vector.reciprocal(q[:], v[:])          # q dead -> reuse as s
            nc.vector.tensor_tensor(nm[:], nm[:], q[:], op=mybir.AluOpType.mult)
            return q, nm                              # (s, t)

        def bn_sync(tiles, tagp):
            """tiles: 4x [128, BL, 12] -> AllReduce'd (s, t) each [128,NC_F,12]."""
            sp = stp.tile([128, NC_F, 2, PATCH], F32, tag=tagp + "sp")
            for c in range(NC_F):
                sq = stp.tile([128, BL, PATCH], F32, tag=tagp + "sq" + str(c), name=tagp + "sq")
                nc.scalar.square(sq[:], tiles[c])
                nc.vector.tensor_reduce(
                    sp[:, c, 0, :], tiles[c].rearrange("f b p -> f p b"),
                    axis=mybir.AxisListType.X, op=mybir.AluOpType.add)
                nc.vector.tensor_reduce(
                    sp[:, c, 1, :], sq.rearrange("f b p -> f p b"),
                    axis=mybir.AxisListType.X, op=mybir.AluOpType.add)
            ci = dram.tile([128, NC_F * 2 * PATCH], F32, tag="ccsin", name="ccsin")
            co = dram.tile([128, NC_F * 2 * PATCH], F32, tag="ccsout", name="ccsout", addr_space="Shared")
            nc.gpsimd.dma_start(ci[:], sp.rearrange("f c k p -> f (c k p)"))
            nc.gpsimd.collective_compute(
                "AllReduce", mybir.AluOpType.add,
                replica_groups=[list(range(NCORES))],
                ins=[ci.opt()], outs=[co.opt()])
            S = stp.tile([128, NC_F, 2, PATCH], F32, tag=tagp + "S", name=tagp + "S", bufs=4)
            nc.gpsimd.dma_start(S.rearrange("f c k p -> f (c k p)"), co[:])
            return scales(S, B, tagp)

        def emit_once():
            # ---- x into SBUF: per f-chunk [128f, 16b, 336t] ----
            for c in range(NC_F):
                nc.sync.dma_start(
                    xb[c][:],
                    x_src[:, c * 128:(c + 1) * 128, :].rearrange("b f t -> f b t"),
                )

            # ============= phase 0: outer BN over [B, F*T] =============
            spack0 = main.tile([128, NC_F, 2, T], F32, tag="spack0", name="spack0")
            TH = T // 2
            for c in range(NC_F):
                nc.vector.tensor_reduce(
                    spack0[:, c, 0, :], xb[c].rearrange("f b t -> f t b"),
                    axis=mybir.AxisListType.X, op=mybir.AluOpType.add)
                for hh in range(2):
                    sq = sqp.tile([128, BL, TH], F32, tag="sq0", name="sq0")
                    xh = xb[c][:, :, hh * TH:(hh + 1) * TH]
                    nc.vector.tensor_mul(sq[:], xh, xh)
                    nc.vector.tensor_reduce(
                        spack0[:, c, 1, hh * TH:(hh + 1) * TH],
                        sq.rearrange("f b t -> f t b"),
                        axis=mybir.AxisListType.X, op=mybir.AluOpType.add)

            # Repack [f, c, k, (w p)] -> [f, w, (c k p)] so each window's
            # stats are a contiguous 96-column block, then AllReduce per
            # window: 28 x 49KB collectives pipeline for ~free while one
            # fused 1.3MB AllReduce costs ~36ms (~38MB/s above ~100KB).
            spackW = main.tile([128, NW, NC_F, 2, PATCH], F32, tag="spackW", name="spackW")
            for k in range(2):
                nc.vector.tensor_copy(
                    spackW[:, :, :, k, :],
                    spack0[:, :, k, :].rearrange("f c (w p) -> f w c p", p=PATCH))
            S0W = main.tile([128, NW, NC_F, 2, PATCH], F32, tag="S0W", name="S0W")
            for w in range(NW):
                ci0 = dram.tile([128, NC_F * 2 * PATCH], F32, tag=f"p0ci{w}", name=f"p0ci{w}")
                co0 = dram.tile([128, NC_F * 2 * PATCH], F32, tag=f"p0co{w}", name=f"p0co{w}",
                                addr_space="Shared")
                nc.gpsimd.dma_start(ci0[:], spackW[:, w, :, :, :].rearrange("f c k p -> f (c k p)"))
                nc.gpsimd.collective_compute(
                    "AllReduce", mybir.AluOpType.add,
                    replica_groups=[list(range(NCORES))],
                    ins=[ci0.opt()], outs=[co0.opt()])
                nc.gpsimd.dma_start(
                    S0W[:, w, :, :, :].rearrange("f c k p -> f (c k p)"), co0[:])
            S0 = main.tile([128, NC_F, 2, T], F32, tag="S0", name="S0")
            for k in range(2):
                nc.vector.tensor_copy(
                    S0[:, :, k, :].rearrange("f c (w p) -> f w c p", p=PATCH),
                    S0W[:, :, :, k, :])

            s0, t0 = scales(S0, B, "bn0")
            # apply outer BN in place: xb = xb*s0 + t0
            for c in range(NC_F):
                nc.vector.tensor_tensor(
                    xb[c][:], xb[c][:],
                    s0[:, c, :].rearrange("f (o t) -> f o t", o=1).broadcast_to((128, BL, T)),
                    op=mybir.AluOpType.mult)
                nc.vector.tensor_tensor(
                    xb[c][:], xb[c][:],
                    t0[:, c, :].rearrange("f (o t) -> f o t", o=1).broadcast_to((128, BL, T)),
                    op=mybir.AluOpType.add)

            # ================= recurrence =================
            for w in range(1, nwin):
                prevs = [xb[c][:, :, (w - 1) * PATCH:w * PATCH] for c in range(NC_F)]
                s1, t1 = bn_sync(prevs, "bn1")

                # t1 -> t1T [12, 512] bf16 (transpose via 32-padded rows so
                # PSUM reads land on 32-aligned partitions), then
                # cT = aggwT.T @ t1T
                t1pad = stp.tile([128, NC_F, 32], F32, tag="t1pad", name="t1pad")
                nc.vector.tensor_copy(t1pad[:, :, 0:PATCH], t1[:])
                t1p = pSm.tile([128, 128], F32, tag="psm", name="psm")
                nc.tensor.transpose(
                    t1p[:], t1pad.rearrange("f c p -> f (c p)"), idf[:])
                t1T = wk.tile([PATCH, F], BF16, tag="t1T", name="t1T")
                for c in range(NC_F):
                    nc.vector.tensor_copy(t1T[:, c * 128:(c + 1) * 128],
                                          t1p[c * 32:c * 32 + PATCH, :])
                cT = pSm.tile([PATCH, F], F32, tag="psm", name="psm")
                nc.tensor.matmul(cT[:], awt[:], t1T[:])

                # rhs tiles [108, 128] per (c, h): rows 0:96 = scaled-prev
                # transposed, rows 96:108 = cT chunk
                R = [[wk.tile([108, 128], BF16, tag=f"R{c}{h}", name=f"R{c}{h}") for h in range(2)]
                     for c in range(NC_F)]
                sc = [wk.tile([128, BL, PATCH], BF16, tag=f"sc{c}", name=f"sc{c}") for c in range(NC_F)]
                for c in range(NC_F):
                    nc.vector.tensor_tensor(
                        sc[c][:], prevs[c],
                        s1[:, c, :].rearrange("f (o p) -> f o p", o=1).broadcast_to(
                            (128, BL, PATCH)),
                        op=mybir.AluOpType.mult)
                for h in range(2):
                    TA = pT.tile([96, F], BF16, tag="pt", name="pt")
                    for c in range(NC_F):
                        nc.tensor.transpose(
                            TA[:, c * 128:(c + 1) * 128],
                            sc[c][:, h * 8:(h + 1) * 8, :].rearrange(
                                "f b p -> f (b p)"),
                            idb[:])
                    for c in range(NC_F):
                        nc.vector.tensor_copy(R[c][h][0:96, :],
                                              TA[:, c * 128:(c + 1) * 128])
                    for c in range(NC_F):
                        nc.vector.tensor_copy(R[c][h][96:108, :],
                                              cT[:, c * 128:(c + 1) * 128])
                # kron matmul + gelu + transpose back
                G = [[wk.tile([96, 128], BF16, tag=f"G{c}{h}", name=f"G{c}{h}") for h in range(2)]
                     for c in range(NC_F)]
                for h in range(2):
                    AG = pT.tile([96, F], F32, tag="pt", name="pt")
                    for c in range(NC_F):
                        nc.tensor.matmul(AG[:, c * 128:(c + 1) * 128],
                                         kron[:], R[c][h][:])
                    for c in range(NC_F):
                        nc.scalar.activation(G[c][h][:], AG[:, c * 128:(c + 1) * 128],
                                             mybir.ActivationFunctionType.Gelu)
                res = [wk.tile([128, BL, PATCH], F32, tag=f"res{c}", name=f"res{c}") for c in range(NC_F)]
                for c in range(NC_F):
                    GT = pBig.tile([128, 2, 96], BF16, tag="pbig", name="pbig")
                    for h in range(2):
                        nc.tensor.transpose(GT[:, h, :], G[c][h][:], idb[0:96, 0:96])
                    nc.vector.tensor_tensor(
                        res[c][:], GT.rearrange("f h x -> f (h x)").rearrange(
                            "f (b p) -> f b p", p=PATCH),
                        xb[c][:, :, w * PATCH:(w + 1) * PATCH],
                        op=mybir.AluOpType.add)

                # ---- BN2 ----
                s2, t2 = bn_sync([r[:] for r in res], "bn2")
                tn = [wk.tile([128, BL, PATCH], BF16, tag=f"tn{c}", name=f"tn{c}") for c in range(NC_F)]
                for c in range(NC_F):
                    tmp = stp.tile([128, BL, PATCH], F32, tag="bn2tmp", name="bn2tmp")
                    nc.vector.tensor_tensor(
                        tmp[:], res[c][:],
                        s2[:, c, :].rearrange("f (o p) -> f o p", o=1).broadcast_to(
                            (128, BL, PATCH)),
                        op=mybir.AluOpType.mult)
                    nc.vector.tensor_tensor(
                        tn[c][:], tmp[:],
                        t2[:, c, :].rearrange("f (o p) -> f o p", o=1).broadcast_to(
                            (128, BL, PATCH)),
                        op=mybir.AluOpType.add)

                # ---- fc1 -> gelu -> fc2 -> gelu ----
                h1 = [wk.tile([128, BL * PATCH], BF16, tag=f"h1{m}", name=f"h1{m}") for m in range(NC_F)]
                for m in range(NC_F):
                    HP = pBig.tile([128, BL * PATCH], F32, tag="pbig", name="pbig")
                    for k in range(NC_F):
                        nc.tensor.matmul(
                            HP[:], w1t[k][:, m * 128:(m + 1) * 128],
                            tn[k].rearrange("f b p -> f (b p)"),
                            start=(k == 0), stop=(k == NC_F - 1))
                    nc.scalar.activation(h1[m][:], HP[:],
                                         mybir.ActivationFunctionType.Gelu)
                for m in range(NC_F):
                    HP = pBig.tile([128, BL * PATCH], F32, tag="pbig", name="pbig")
                    for k in range(NC_F):
                        nc.tensor.matmul(
                            HP[:], w2t[k][:, m * 128:(m + 1) * 128], h1[k][:],
                            start=(k == 0), stop=(k == NC_F - 1))
                    g = wk.tile([128, BL * PATCH], BF16, tag="gg", name="gg", bufs=4)
                    nc.scalar.activation(g[:], HP[:],
                                         mybir.ActivationFunctionType.Gelu)
                    g5 = wk.tile([128, BL * PATCH], BF16, tag="gg5", name="gg5")
                    nc.vector.tensor_scalar_mul(g5[:], g[:], ALPHA)
                    nc.vector.tensor_tensor(
                        xb[m][:, :, w * PATCH:(w + 1) * PATCH],
                        g5.rearrange("f (b p) -> f b p", p=PATCH),
                        res[m][:], op=mybir.AluOpType.add)

            # ---- write out ----
            for c in range(NC_F):
                nc.sync.dma_start(
                    y_dst[:, c * 128:(c + 1) * 128, :].rearrange("b f t -> f b t"),
                    xb[c][:, :, :])

        for _rep in range(reps):
            emit_once()
        if timing:
            nc.sync.dma_start(y_d[:], xb[0][:, 0, 0:16])
    return nc


def _make_runner(nc, n_cores=NCORES):
    """jit the shard_map once; return a callable in_maps -> dict of global outs.

    Mirrors bass2jax.run_bass_via_pjrt but keeps the jitted function alive so
    repeat calls skip retrace/relower/recompile (which costs ~0.5s per call
    for this NEFF under axon). Non-x inputs are device_put once and reused.
    """
    install_neuronx_cc_hook()
    partition_name = nc.partition_id_tensor.name if nc.partition_id_tensor else None
    in_names, out_names, out_avals = [], [], []
    for alloc in nc.m.functions[0].allocations:
        if not isinstance(alloc, mybir.MemoryLocationSet):
            continue
        name = alloc.memorylocations[0].name
        if alloc.kind == "ExternalInput":
            if name != partition_name:
                in_names.append(name)
        elif alloc.kind == "ExternalOutput":
            out_names.append(name)
            shape = tuple(alloc.tensor_shape)
            dtype = mybir.dt.np(alloc.dtype)
            out_avals.append(jax.core.ShapedArray(shape, dtype))
    n_params = len(in_names)
    n_outs = len(out_avals)
    in_names_all = in_names + out_names + ([partition_name] if partition_name else [])
    donate = tuple(range(n_params, n_params + n_outs))

    def _body(*args):
        operands = list(args)
        if partition_name is not None:
            operands.append(partition_id_tensor())
        outs = _bass_exec_p.bind(
            *operands, out_avals=tuple(out_avals), in_names=tuple(in_names_all),
            out_names=tuple(out_names), lowering_input_output_aliases=(),
            sim_require_finite=True, sim_require_nnan=True, nc=nc)
        return tuple(outs)

    devices = jax.devices()[:n_cores]
    mesh = Mesh(np.asarray(devices), ("core",))
    in_specs = (PartitionSpec("core"),) * (n_params + n_outs)
    out_specs = (PartitionSpec("core"),) * len(out_names)
    sharded = jax.jit(shard_map(_body, mesh=mesh, in_specs=in_specs,
                                out_specs=out_specs, check_rep=False),
                      donate_argnums=donate, keep_unused=True)

    const_cache = {}
    # Donated output buffers are created on-device (an XLA zeros program)
    # instead of shipping n_cores x 11MB of host zeros through the axon
    # tunnel every call.
    import jax.numpy as jnp
    zero_makers = [
        jax.jit(
            (lambda gs, dt: (lambda: jnp.zeros(gs, dt)))(
                (n_cores * a.shape[0], *a.shape[1:]), a.dtype),
            out_shardings=NamedSharding(mesh, PartitionSpec("core")))
        for a in out_avals
    ]

    def run(in_maps):
        per_core = [[np.asarray(m[name]) for name in in_names] for m in in_maps]
        concat_in = []
        for i, name in enumerate(in_names):
            arr = np.concatenate([per_core[c][i] for c in range(n_cores)], axis=0)
            if name != "x":
                h = hash(arr.tobytes())
                hit = const_cache.get(name)
                if hit is None or hit[0] != h:
                    hit = (h, jax.device_put(
                        arr, NamedSharding(mesh, PartitionSpec("core"))))
                    const_cache[name] = hit
                arr = hit[1]
            concat_in.append(arr)
        concat_zeros = [zm() for zm in zero_makers]
        out = sharded(*concat_in, *concat_zeros)
        return {name: np.asarray(o) for name, o in zip(out_names, out)}

    return run


_CACHE = {}


def _get_runner():
    if "run" not in _CACHE:
        nc = bacc.Bacc()
        _build(nc, NW)
        nc.compile()
        _CACHE["run"] = _make_runner(nc)
    return _CACHE["run"]


def _weight_inputs(inputs):
    agg_w = np.asarray(inputs["agg_w"], np.float32)
    fc1_w = np.asarray(inputs["fc1_w"], np.float32)
    fc2_w = np.asarray(inputs["fc2_w"], np.float32)

    bf = ml_dtypes.bfloat16
    w1t = fc1_w.T.astype(bf)                      # [F, FF]
    w2t = fc2_w.T.astype(bf)                      # [FF, F]
    kron = np.zeros((108, 96), np.float32)
    for b in range(8):
        for p in range(PATCH):
            for q in range(PATCH):
                kron[b * PATCH + p, b * PATCH + q] = agg_w[q, p]
    for q in range(PATCH):
        kron[96 + q, np.arange(8) * PATCH + q] = 1.0
    return {
        "w1t": w1t, "w2t": w2t, "kron": kron.astype(bf),
        "awt": np.ascontiguousarray(agg_w.T).astype(bf),
        "idf": np.eye(128, dtype=np.float32),
        "idb": np.eye(128).astype(bf),
    }


def kernel(**inputs):
    x = np.asarray(inputs["x"], np.float32)
    wmap = _weight_inputs(inputs)
    run = _get_runner()

    in_maps = []
    for i in range(NCORES):
        in_maps.append({"x": np.ascontiguousarray(x[i * BL:(i + 1) * BL]), **wmap})
    _t0 = _time.time()
    out = run(in_maps)
    global LAST_RUN_WALL
    LAST_RUN_WALL = _time.time() - _t0
    print("run_bass_kernel_spmd wall: %.3fs" % LAST_RUN_WALL)
    y = out["y"]                                  # global [B, F, T]
    return np.ascontiguousarray(y.astype(np.float32))
